# revision 15
# baseline (speedup 1.0000x reference)
"""DTNN layer kernel for Trainium2 (8 NeuronCores).

Math: out[b,i,o] = sum_j sum_h Wfc[o,h] * hx[b,i,h] * hd[b,i,j,h]
with hx = x@Wcf.T + bcf, hd = dist@Wdf.T + bdf.
Since Wfc/Wdf are linear, the j-sum commutes:
    ds[b,i,d]  = sum_j dist[b,i,j,d]                  (memory-bound reduction)
    out[b,i,:] = ((x@Wcf.T + bcf) * (ds@Wdf.T + N*bdf)) @ Wfc.T
So the kernel streams `distance` once (134MB) and does a few 128x128 matmuls.

Sharding: flatten (B,N) -> 1024 i-rows, 128 rows per core; no cross-core comms.

Schedule (from NTFF traces of the 70us/69us/67us predecessors):
- dist streams on the sync HWDGE queue as 15 tapered tiles
  [8,16,32x5,16,16,16,8,8,4,2,2]; a small tile leads so folding starts
  ~12us, and the taper ends at 2j so the last fold is ~0.3us.  The two
  constant blocks ride 2nd/3rd on the same queue -- on the scalar queue
  their contiguous-source packets aggregated onto one DMA engine and landed
  at t=29us, stalling the whole in-order DVE program (the original 70us
  bug).  A leftover probe DMA on the scalar queue (unused result) showed
  strided sources stripe across all 16 engines there; an alternating
  dual-queue stream variant measured slower, so single-queue stands.
- Each tile folds to 128 columns by halving adds on DVE as it lands.  Level
  1 reads the f32 stream and writes bf16; levels >=2 are pure-bf16
  tensor_tensor adds which hit the DVE 2x_1p mode (2 elem/lane/cycle),
  cutting fold time ~35%.  All folds stay on DVE: a GpSimd-assist variant
  ran both engines ~2x slower from SBUF contention.
- PE transpose-accumulates each bf16 partial into a PSUM dsT bank (bf16
  matmul vs identity, ~0.4us single-pass; fp32 matmuls are dual-pass
  LOW_HIGH at ~1.6us).  This removes accumulator adds and the tail
  transpose entirely.
- Tail: last fold -> transpose-accum(stop) -> dsT copy (cast bf16) ->
  hd matmul -> sT mul -> out matmul (accumulates onto PSUM preloaded with
  the (hx*N*bdf)@WfcT bias term during the stream) -> copy -> DMA out.
- Numerics: bf16 fold partials + bf16 tail matmuls give rel err ~4e-3
  against the f32 reference (gate is 2e-2).
"""

import numpy as np
from ml_dtypes import bfloat16

import concourse.bass as bass
import concourse.bacc as bacc
import concourse.mybir as mybir
from concourse.tile import TileContext
from concourse.bass_utils import run_bass_kernel_spmd

B, N, D, H = 4, 256, 128, 128
NCORES = 8
ROWS = B * N // NCORES  # 128 i-rows per core
FP = mybir.dt.float32
BF = mybir.dt.bfloat16

# dist tile taper (j-columns per DMA); constants ride after the first two
SIZES = [8, 16, 32, 32, 32, 32, 32, 16, 16, 16, 8, 8, 4, 2, 2]
CST_AFTER = 1  # number of dist tiles DMA'd before the constant blocks
assert sum(SIZES) == N

# f32 constant block columns: [xT | wcfT | bcf_col | bdf_col]
CF_XT = 0
CF_WCF = 128
CF_BCF = 256
CF_BDF = 257
CF_TOT = 258
# bf16 constant block columns: [wdfT | wfcT | eye]
CB_WDF = 0
CB_WFC = 128
CB_EYE = 256
CB_TOT = 384


def build_nc():
    nc = bacc.Bacc("TRN2", target_bir_lowering=False)
    dist = nc.declare_dram_parameter("dist", [ROWS, N * D], FP, isOutput=False)
    cstf = nc.declare_dram_parameter("cstf", [128, CF_TOT], FP, isOutput=False)
    cstb = nc.declare_dram_parameter("cstb", [128, CB_TOT], BF, isOutput=False)
    out = nc.declare_dram_parameter("out", [ROWS, D], FP, isOutput=True)

    with TileContext(nc) as tc:
        with (
            tc.tile_pool(name="const", bufs=1) as cpool,
            tc.tile_pool(name="dist", bufs=1) as dpool,
            tc.tile_pool(name="work", bufs=1) as wpool,
            tc.tile_pool(name="psum", bufs=1, space="PSUM") as ppool,
        ):
            # DMA order on the sync queue: two small dist tiles (folding can
            # start ~15us), then the constants, then the rest of the stream.
            dtiles = []
            cstf_t = cstb_t = None
            off = 0
            for k, jn in enumerate(SIZES):
                if k == CST_AFTER:
                    cstf_t = cpool.tile([128, CF_TOT], FP, tag="cstf",
                                        name="cstf_t")
                    nc.sync.dma_start(out=cstf_t[:], in_=cstf[:])
                    cstb_t = cpool.tile([128, CB_TOT], BF, tag="cstb",
                                        name="cstb_t")
                    nc.sync.dma_start(out=cstb_t[:], in_=cstb[:])
                t = dpool.tile([ROWS, jn * D], FP, tag=f"dist{k}",
                               name=f"dist{k}_t")
                nc.sync.dma_start(out=t[:], in_=dist[:, off * D:(off + jn) * D])
                dtiles.append(t)
                off += jn

            # Probe: one 64KB read on the scalar-engine HWDGE queue, result
            # unused.  The trace shows whether big descriptors stripe across
            # the 16 DMA engines on this queue (the 4KB-descriptor constant
            # block famously did not) -- informs a dual-queue stream variant.
            probe_t = cpool.tile([ROWS, D], FP, tag="probe", name="probe_t")
            nc.scalar.dma_start(out=probe_t[:], in_=dist[:, 0:D])

            xT_t = cstf_t[:, CF_XT:CF_XT + ROWS]
            wcf_t = cstf_t[:, CF_WCF:CF_WCF + H]
            bcf_col = cstf_t[:, CF_BCF:CF_BCF + 1]
            bdf_col = cstf_t[:, CF_BDF:CF_BDF + 1]
            wdf_b = cstb_t[:, CB_WDF:CB_WDF + H]
            wfc_b = cstb_t[:, CB_WFC:CB_WFC + D]
            eye_b = cstb_t[:, CB_EYE:CB_EYE + ROWS]

            # hx^T = (Wcf^T)^T @ x^T -> (H, ROWS) in PSUM (fp32)
            hx_ps = ppool.tile([H, ROWS], FP, tag="hx")
            nc.tensor.matmul(hx_ps[:], wcf_t, xT_t, start=True, stop=True)

            # bf16 halving scratch per tile (level-1 output and below)
            folds = [wpool.tile([ROWS, max(SIZES[k] // 2, 1) * D], BF,
                                tag=f"fold{k}", name=f"fold{k}_t")
                     for k in range(len(SIZES))]

            def emit_fold(k):
                # level 1: f32 tile halves -> bf16 scratch; levels >= 2:
                # bf16 in-place halving (DVE 2x_1p mode)
                t, fb, jn = dtiles[k], folds[k], SIZES[k]
                half = jn * D // 2
                nc.vector.tensor_add(fb[:, 0:half], t[:, 0:half],
                                     t[:, half:2 * half])
                while half > D:
                    h2 = half // 2
                    nc.vector.tensor_add(fb[:, 0:h2], fb[:, 0:h2],
                                         fb[:, h2:half])
                    half = h2

            emit_fold(0)

            # hx^T + bcf (f32), s0T = hxT * bdf * N (bf16, cast on write)
            hxT = wpool.tile([H, ROWS], FP, tag="hxT")
            nc.vector.tensor_scalar(hxT[:], hx_ps[:], bcf_col, None,
                                    mybir.AluOpType.add)
            s0T = wpool.tile([H, ROWS], BF, tag="s0T")
            nc.vector.tensor_scalar(s0T[:], hxT[:], bdf_col, float(N),
                                    mybir.AluOpType.mult,
                                    mybir.AluOpType.mult)

            for k in range(1, len(SIZES)):
                emit_fold(k)

            # Preload the bias term (hx * N*bdf) @ Wfc^T into the output
            # PSUM during the stream (bf16 matmul).
            out_ps = ppool.tile([ROWS, D], FP, tag="out")
            nc.tensor.matmul(out_ps[:], s0T[:], wfc_b, start=True, stop=False,
                             skip_group_check=True)

            # PE transpose-accumulate each bf16 partial into dsT PSUM:
            # dsT_ps += fold_k^T @ eye
            dsT_ps = ppool.tile([D, ROWS], FP, tag="dsT")
            for k in range(len(SIZES)):
                nc.tensor.matmul(dsT_ps[:], folds[k][:, 0:D], eye_b,
                                 start=(k == 0), stop=(k == len(SIZES) - 1),
                                 skip_group_check=True)

            dsT = wpool.tile([D, ROWS], BF, tag="dsTb")
            nc.vector.tensor_copy(dsT[:], dsT_ps[:])

            # hd^T (bias-free) = (Wdf^T)^T @ ds^T -> (H, ROWS), bf16 matmul
            hd_ps = ppool.tile([H, ROWS], FP, tag="hd")
            nc.tensor.matmul(hd_ps[:], wdf_b, dsT[:], start=True, stop=True,
                             skip_group_check=True)

            # s^T = hx^T * hd^T (bf16 out, cast on write)
            sT = wpool.tile([H, ROWS], BF, tag="sT")
            nc.vector.tensor_mul(sT[:], hd_ps[:], hxT[:])

            # out += sT^T @ Wfc^T, accumulating onto the preloaded bias term.
            # Split into row-halves so the first half's DMA trigger (~1.2us
            # HWDGE launch latency) overlaps the second half's matmul+copy.
            half_r = ROWS // 2
            out_sbA = wpool.tile([half_r, D], FP, tag="outsbA")
            out_sbB = wpool.tile([half_r, D], FP, tag="outsbB")
            nc.tensor.matmul(out_ps[0:half_r, :], sT[:, 0:half_r], wfc_b,
                             start=False, stop=True, skip_group_check=True)
            nc.vector.tensor_copy(out_sbA[:], out_ps[0:half_r, :])
            nc.sync.dma_start(out=out[0:half_r, :], in_=out_sbA[:])
            nc.tensor.matmul(out_ps[half_r:ROWS, :], sT[:, half_r:ROWS],
                             wfc_b, start=False, stop=True,
                             skip_group_check=True)
            nc.vector.tensor_copy(out_sbB[:], out_ps[half_r:ROWS, :])
            nc.sync.dma_start(out=out[half_r:ROWS, :], in_=out_sbB[:])
    nc.compile()
    return nc


_NC_CACHE = None


def _get_nc():
    global _NC_CACHE
    if _NC_CACHE is None:
        _NC_CACHE = build_nc()
    return _NC_CACHE


def _make_in_maps(x, distance, Wcf_w, Wcf_b, Wdf_w, Wdf_b, Wfc_w):
    x = np.ascontiguousarray(np.asarray(x, np.float32))
    distance = np.ascontiguousarray(np.asarray(distance, np.float32))
    x_flat = x.reshape(B * N, D)
    dist_flat = distance.reshape(B * N, N * D)
    wcfT = np.asarray(Wcf_w, np.float32).T
    bcf = np.asarray(Wcf_b, np.float32)
    bdf = np.asarray(Wdf_b, np.float32)
    cstb = np.zeros((128, CB_TOT), bfloat16)
    cstb[:, CB_WDF:CB_WDF + H] = np.asarray(Wdf_w, np.float32).T.astype(bfloat16)
    cstb[:, CB_WFC:CB_WFC + D] = np.asarray(Wfc_w, np.float32).T.astype(bfloat16)
    cstb[:, CB_EYE:CB_EYE + ROWS] = np.eye(ROWS, dtype=np.float32).astype(bfloat16)
    in_maps = []
    for c in range(NCORES):
        sl = slice(c * ROWS, (c + 1) * ROWS)
        cstf = np.zeros((128, CF_TOT), np.float32)
        cstf[:, CF_XT:CF_XT + ROWS] = x_flat[sl].T
        cstf[:, CF_WCF:CF_WCF + H] = wcfT
        cstf[:, CF_BCF] = bcf
        cstf[:, CF_BDF] = bdf
        in_maps.append({
            "dist": np.ascontiguousarray(dist_flat[sl]),
            "cstf": cstf,
            "cstb": cstb,
        })
    return in_maps


def kernel(x, distance, Wcf_w, Wcf_b, Wdf_w, Wdf_b, Wfc_w):
    in_maps = _make_in_maps(x, distance, Wcf_w, Wcf_b, Wdf_w, Wdf_b, Wfc_w)
    nc = _get_nc()
    res = run_bass_kernel_spmd(nc, in_maps, list(range(NCORES))).results
    out = np.concatenate([res[c]["out"] for c in range(NCORES)], axis=0)
    return out.reshape(B, N, D)
